# revision 36
# baseline (speedup 1.0000x reference)
"""Trainium2 Bass kernel: causal pre-LN attention block, SPMD on 8 NeuronCores.

Sharding: core c handles batch b = c//4 and heads [4*(c%4), 4*(c%4)+4).
Each core computes LN + QKV projection for its heads + causal attention +
a partial output projection (contracting only its heads); the host sums the
4 partials per batch (the tensor-parallel reduce) and adds b_out.

Everything on device is kept "transposed" (features on partitions) so no
on-device transposes are needed:
  xT [D, S] -> LN stats via PE matmul-with-ones -> xn (bf16)
  qT,kT [64, S] per head (features on partitions), v [S, 64] (tokens on
  partitions, with a ones column appended so the attention matmul also
  produces softmax denominators)
  S^T = kT.T @ qT  [k, q]  (fp32r), exp on ACT (scores are ~N(0,1) after
  LN so no max subtraction is needed; causal masking via narrowed matmuls
  + memset + a triangular multiply)
  attnU^T = v_aug.T @ P^T accumulated in PSUM; normalize with broadcast 1/l
  out_partial = A^T.T @ w_out (fp32r)
"""

import sys

import numpy as np

sys.path.insert(0, "/opt/trn_rl_repo")

import ml_dtypes  # noqa: E402

B, S, D, H, HD = 2, 2048, 1024, 16, 64
LN_EPS = 1e-6
NCORES = 8
GH = 4  # heads per core
DC = 8  # 128-row chunks of D
ST = 16  # 128-row s tiles
QT = 4  # 512-wide q tiles
P = 128
NQ = 512

_BF16 = ml_dtypes.bfloat16

_cache: dict = {}


def _build(causal: bool, qk_bias: bool, tri_on_gpsimd: bool = False,
           exp_batched: bool = True, av_narrow: bool = True, fast_recip: bool = False):
    from contextlib import ExitStack

    import concourse.bacc as bacc
    import concourse.bass as bass
    import concourse.tile as tile
    from concourse import mybir

    f32 = mybir.dt.float32
    f32r = mybir.dt.float32r
    bf16 = mybir.dt.bfloat16
    AF = mybir.ActivationFunctionType
    OP = mybir.AluOpType

    def bcast(src_ap, n):
        """Partition-broadcast AP: replicate a 1-partition source across n."""
        return bass.AP(
            tensor=src_ap.tensor,
            offset=src_ap.offset,
            ap=[[0, n]] + [list(e) for e in src_ap.ap[1:]],
        )

    nc = bacc.Bacc("TRN2", target_bir_lowering=False, debug=False)
    d_xT = nc.declare_dram_parameter("xT", [D, S], bf16, isOutput=False)
    # q (and k) weights packed two-heads-per-128 so that for head h both qT
    # and kT live at partitions [(h%2)*64, (h%2)*64+64) (PE needs equal base
    # partitions for both matmul operands).
    d_wq = nc.declare_dram_parameter("wq", [P, DC, 2, P], bf16, isOutput=False)
    d_wk = nc.declare_dram_parameter("wk", [P, DC, 2, P], bf16, isOutput=False)
    d_wv = nc.declare_dram_parameter("wv", [P, DC, GH, HD], bf16, isOutput=False)
    d_wo = nc.declare_dram_parameter("wo", [P, 2, D], bf16, isOutput=False)
    d_tri = nc.declare_dram_parameter("tri", [P, P], bf16, isOutput=False)
    if qk_bias:
        d_qkb = nc.declare_dram_parameter("qkb", [1, 2, 2, P], bf16, isOutput=False)
    if not causal:
        d_maskT = nc.declare_dram_parameter("maskT", [S, S], bf16, isOutput=False)
    d_out = nc.declare_dram_parameter("out", [S, D], f32, isOutput=True)

    mask_eng = nc.gpsimd if tri_on_gpsimd else nc.vector

    with tile.TileContext(nc) as tc, ExitStack() as ctx:
        con = ctx.enter_context(tc.tile_pool(name="con", bufs=1))
        ps_sc = ctx.enter_context(tc.tile_pool(name="ps_sc", bufs=2, space="PSUM"))
        ps_acc = ctx.enter_context(tc.tile_pool(name="ps_acc", bufs=2, space="PSUM"))
        ps_mm = ctx.enter_context(tc.tile_pool(name="ps_mm", bufs=2, space="PSUM"))
        x2p = ctx.enter_context(tc.tile_pool(name="x2p", bufs=2))
        rowp = ctx.enter_context(tc.tile_pool(name="rowp", bufs=2))
        pp = ctx.enter_context(tc.tile_pool(name="pp", bufs=4))
        rlp = ctx.enter_context(tc.tile_pool(name="rlp", bufs=2))
        rlsp = ctx.enter_context(tc.tile_pool(name="rlsp", bufs=2))
        outp = ctx.enter_context(tc.tile_pool(name="outp", bufs=2))
        drp = ctx.enter_context(tc.tile_pool(name="drp", bufs=2, space="DRAM"))
        mkp = (
            ctx.enter_context(tc.tile_pool(name="mkp", bufs=3))
            if not causal
            else None
        )

        # ---- persistent tiles ----
        xt_sb = con.tile([P, DC, S], bf16, tag="xt")
        xn_sb = con.tile([P, DC, S], bf16, tag="xn")
        wq_sb = con.tile([P, DC, 2, P], bf16, tag="wq")
        wk_sb = con.tile([P, DC, 2, P], bf16, tag="wk")
        wv_sb = con.tile([P, DC, GH, HD], bf16, tag="wv")
        wo_sb = con.tile([P, 2, D], bf16, tag="wo")
        tri_sb = con.tile([P, P], bf16, tag="tri")
        qT_sb = con.tile([P, 2, S], bf16, tag="qT")
        kT_sb = con.tile([P, 2, S], bf16, tag="kT")
        v_sb = con.tile([P, ST, GH, HD + 1], bf16, tag="v")
        a_sb = con.tile([P, 2, S], bf16, tag="a")
        ashp = ctx.enter_context(tc.tile_pool(name="ashp", bufs=2))
        mu_b = con.tile([P, S], bf16, tag="mu_b")
        rstd_b = con.tile([P, S], bf16, tag="rstd_b")
        ones_sb = con.tile([P, 1], bf16, tag="ones")
        onesd_sb = con.tile([P, NQ], f32, tag="onesd")
        eps_sb = con.tile([1, 1], f32, tag="eps")
        if qk_bias:
            onesrow_sb = con.tile([1, NQ], bf16, tag="onesrow")
            qkb_sb = con.tile([1, 2, 2, P], bf16, tag="qkb")

        nc.vector.memset(ones_sb[:, :], 1.0)
        nc.vector.memset(onesd_sb[:, :], 1.0)
        nc.vector.memset(eps_sb[:, :], LN_EPS)
        # fill v wholesale with ones; v evictions overwrite cols [0:HD], leaving
        # a ones column at HD so the attention matmul also produces l
        nc.vector.memset(v_sb[:, :, :, :], 1.0)
        if qk_bias:
            nc.vector.memset(onesrow_sb[:, :], 1.0)
            nc.sync.dma_start(out=qkb_sb[:, :, :, :], in_=d_qkb[:, :, :, :])

        # ---- weight + const loads ----
        nc.sync.dma_start(out=tri_sb[:, :], in_=d_tri[:, :])
        nc.sync.dma_start(out=wq_sb[:, :, :, :], in_=d_wq[:, :, :, :])
        nc.sync.dma_start(out=wk_sb[:, :, :, :], in_=d_wk[:, :, :, :])
        nc.sync.dma_start(out=wv_sb[:, :, :, :], in_=d_wv[:, :, :, :])
        nc.sync.dma_start(out=wo_sb[:, :, :], in_=d_wo[:, :, :])

        # ---- xT loads, chunk-major, split across two DMA paths ----
        xT_r = d_xT.rearrange("(c p) s -> p c s", p=P)
        for c in range(DC):
            eng = (nc.sync, nc.gpsimd, nc.sync, nc.gpsimd)[c % 4]
            eng.dma_start(out=xt_sb[:, c, :], in_=xT_r[:, c, :])

        # ---- LN stats per 512-col slice: mu, rstd rows + partition broadcast ----
        for q4 in range(QT):
            sl = slice(q4 * NQ, (q4 + 1) * NQ)
            mu_ps = ps_acc.tile([1, NQ], f32, tag="acc")
            msq_ps = ps_acc.tile([1, NQ], f32, tag="acc")
            for c in range(DC):
                x2t = x2p.tile([P, NQ], bf16, tag="x2")
                nc.vector.tensor_mul(x2t[:, :], xt_sb[:, c, sl], xt_sb[:, c, sl])
                nc.tensor.matmul(
                    mu_ps[:, :], ones_sb[:, :], xt_sb[:, c, sl],
                    start=(c == 0), stop=(c == DC - 1),
                )
                nc.tensor.matmul(
                    msq_ps[:, :], ones_sb[:, :], x2t[:, :],
                    start=(c == 0), stop=(c == DC - 1),
                )
            # mu row (bf16) and rstd row (f32 chain then bf16)
            nc.scalar.activation(mu_b[0:1, sl], mu_ps[:, :], AF.Copy, scale=1.0 / D)
            r0 = rowp.tile([1, NQ], f32, tag="r0")
            mu2 = rowp.tile([1, NQ], f32, tag="mu2")
            nc.scalar.activation(r0[:, :], msq_ps[:, :], AF.Copy, scale=1.0 / D)
            nc.vector.tensor_mul(mu2[:, :], mu_b[0:1, sl], mu_b[0:1, sl])
            nc.vector.tensor_sub(r0[:, :], r0[:, :], mu2[:, :])
            nc.scalar.activation(r0[:, :], r0[:, :], AF.Sqrt, bias=eps_sb[:, :])
            nc.vector.reciprocal(r0[:, :], r0[:, :])
            nc.scalar.activation(rstd_b[0:1, sl], r0[:, :], AF.Copy)
            # broadcast row 0 across remaining partitions (step-0 partition
            # APs are only legal on the DRAM side, so bounce through DRAM)
            mu_d = drp.tile([1, NQ], bf16, tag="mud")
            rs_d = drp.tile([1, NQ], bf16, tag="rsd")
            nc.scalar.dma_start(out=mu_d[:, :], in_=mu_b[0:1, sl])
            nc.scalar.dma_start(out=mu_b[1:P, sl], in_=bcast(mu_d[:, :], P - 1))
            nc.scalar.dma_start(out=rs_d[:, :], in_=rstd_b[0:1, sl])
            nc.scalar.dma_start(out=rstd_b[1:P, sl], in_=bcast(rs_d[:, :], P - 1))
            # xn = (xT - mu) * rstd for this slice (all chunks)
            for c in range(DC):
                nc.vector.tensor_sub(xn_sb[:, c, sl], xt_sb[:, c, sl], mu_b[:, sl])
                nc.vector.tensor_mul(xn_sb[:, c, sl], xn_sb[:, c, sl], rstd_b[:, sl])

        def emit_outproj(qt, only_st=None):
            for st in ([only_st] if only_st is not None else range(4 * qt, 4 * qt + 4)):
                ssl = slice(st * P, (st + 1) * P)
                for fb in range(2):
                    fsl = slice(fb * NQ, (fb + 1) * NQ)
                    o_ps = ps_mm.tile([P, NQ], f32, tag="mm")
                    for c2 in range(2):
                        nc.tensor.matmul(
                            o_ps[:, :], a_sb[:, c2, ssl], wo_sb[:, c2, fsl],
                            start=(c2 == 0), stop=(c2 == 1),
                        )
                    ot = outp.tile([P, NQ], f32, tag="out")
                    nc.vector.tensor_copy(ot[:, :], o_ps[:, :])
                    nc.sync.dma_start(out=d_out[ssl, fsl], in_=ot[:, :])

        # ---- main loop over q blocks ----
        for qt in range(QT):
            qsl = slice(qt * NQ, (qt + 1) * NQ)
            # v for the 4 s-tiles of this block
            for st in range(4 * qt, 4 * qt + 4):
                ssl = slice(st * P, (st + 1) * P)
                v_ps = ps_mm.tile([P, GH, HD], f32, tag="mm")
                for c in range(DC):
                    nc.tensor.matmul(
                        v_ps[:, :, :], xn_sb[:, c, ssl], wv_sb[:, c, :, :],
                        start=(c == 0), stop=(c == DC - 1),
                    )
                nc.scalar.activation(v_sb[:, st, :, 0:HD], v_ps[:, :, :], AF.Copy)
            # qT/kT, two heads packed per matmul (pair pr = h//2)
            for qk in range(2):
                w_sb, dst = (wq_sb, qT_sb) if qk == 0 else (wk_sb, kT_sb)
                for pr in range(2):
                    qk_ps = ps_mm.tile([P, NQ], f32, tag="mm")
                    for c in range(DC):
                        nc.tensor.matmul(
                            qk_ps[:, :], w_sb[:, c, pr, :], xn_sb[:, c, qsl],
                            start=(c == 0), stop=(c == DC - 1) and not qk_bias,
                        )
                    if qk_bias:
                        nc.tensor.matmul(
                            qk_ps[:, :], qkb_sb[:, qk, pr, :], onesrow_sb[:, :],
                            start=False, stop=True,
                        )
                    nc.vector.tensor_copy(dst[:, pr, qsl], qk_ps[:, :])

            # attention for each head
            for h in range(GH):
                even = h % 2 == 0
                pr = h // 2
                r0h = (h % 2) * HD  # partition base of this head's qT/kT rows
                acc = ps_acc.tile([P, NQ], f32, tag="acc")
                kts = list(range(4 * qt + 4)) if causal else list(range(ST))
                n_kt = len(kts)
                c0s = {}
                for g0 in range(0, n_kt, 2):
                    grp = kts[g0 : g0 + 2]
                    st_ps = ps_sc.tile([P, 2, NQ], f32, tag="sc")
                    pt = pp.tile([P, 2, NQ], bf16, tag="p")
                    mt = mkp.tile([P, 2, NQ], bf16, tag="mk") if not causal else None
                    for i, kt in enumerate(grp):
                        j = kt - 4 * qt  # >=0 on the diagonal band (causal)
                        ksl = slice(kt * P, (kt + 1) * P)
                        if causal and j >= 0:
                            c0 = min(j * P, 2 * P)  # keep moving width >= 256
                        else:
                            c0 = 0
                        c0s[kt] = c0
                        nc.tensor.matmul(
                            st_ps[:, i, c0:NQ],
                            kT_sb[r0h : r0h + HD, pr, ksl],
                            qT_sb[r0h : r0h + HD, pr, qt * NQ + c0 : (qt + 1) * NQ],
                            start=True, stop=True,
                        )
                    # one exp over the whole group; garbage in the unused
                    # region [0:c0] is never read by the narrowed AV matmul
                    if exp_batched:
                        nc.scalar.activation(
                            pt[:, :, :], st_ps[:, :, :], AF.Exp, scale=0.125
                        )
                    else:
                        for i, kt in enumerate(grp):
                            c0 = c0s[kt]
                            nc.scalar.activation(
                                pt[:, i, c0:NQ], st_ps[:, i, c0:NQ], AF.Exp, scale=0.125
                            )
                    if not causal:
                        nc.sync.dma_start(
                            out=mt[:, :, :],
                            in_=d_maskT.rearrange("(kt p) s -> p kt s", p=P)[
                                :, grp[0] : grp[0] + 2, qsl
                            ],
                        )
                        nc.vector.tensor_mul(pt[:, :, :], pt[:, :, :], mt[:, :, :])
                    for i, kt in enumerate(grp):
                        j = kt - 4 * qt
                        if causal and j >= 0:
                            m0 = j * P  # first valid column of this band block
                            if m0 > c0s[kt]:
                                nc.vector.memset(pt[:, i, c0s[kt] : m0], 0.0)
                            mask_eng.tensor_mul(
                                pt[:, i, m0 : m0 + P], pt[:, i, m0 : m0 + P],
                                tri_sb[:, :],
                            )
                    for i, kt in enumerate(grp):
                        c0 = c0s[kt] if av_narrow else 0
                        if not av_narrow and c0s[kt] < (kt - 4 * qt) * P:
                            nc.vector.memset(pt[:, i, c0s[kt] : (kt - 4 * qt) * P], 0.0)
                        nc.tensor.matmul(
                            acc[0 : HD + 1, c0:NQ], v_sb[:, kt, h, :], pt[:, i, c0:NQ],
                            start=(kt == kts[0]), stop=(kt == kts[-1]),
                        )
                # normalize: rl = 1/l broadcast across the 64 head dims
                rls = rlsp.tile([P, 2, NQ], f32, tag="rls")
                rl_b = rlp.tile([P, NQ], f32, tag="rlb")
                lrow = slice(HD, HD + 1)
                nc.vector.tensor_copy(rls[lrow, 0, :], acc[lrow, :])
                rl_d = drp.tile([1, NQ], f32, tag="rld")
                nc.gpsimd.dma_start(out=rl_d[:, :], in_=rls[lrow, 0, :])
                rrows = slice(0, HD)
                nc.gpsimd.dma_start(out=rl_b[rrows, :], in_=bcast(rl_d[:, :], HD))
                nc.vector.reciprocal(rl_b[rrows, :], rl_b[rrows, :])
                if even:
                    nc.vector.tensor_mul(
                        a_sb[0:HD, h // 2, qsl], acc[0:HD, :], rl_b[0:HD, :]
                    )
                else:
                    # engines cannot shift partitions; bounce through a DMA
                    ash = ashp.tile([HD, NQ], bf16, tag="ash")
                    nc.vector.tensor_mul(ash[:, :], acc[0:HD, :], rl_b[0:HD, :])
                    nc.gpsimd.dma_start(
                        out=a_sb[HD : 2 * HD, h // 2, qsl], in_=ash[:, :]
                    )
                # deferred output projection piece: fills PE stalls while the
                # next head's attention waits on exp
                if qt >= 1:
                    emit_outproj(qt - 1, only_st=4 * (qt - 1) + h)

        emit_outproj(QT - 1)

    nc.compile()
    return nc


def _host_prep(x, ln_scale, ln_bias, w_qkv, b_qkv, w_out, b_out, mask):
    """Fold LN affine params and exactly-absorbable biases on the host."""
    f32 = np.float32
    wq = (ln_scale.astype(f32)[:, None, None] * w_qkv.astype(f32))  # [D, H, 192]
    bias_eff = ln_bias.astype(f32) @ w_qkv.reshape(D, -1).astype(f32)
    bias_eff = bias_eff.reshape(H, 3 * HD) + b_qkv.astype(f32)  # [H, 192]
    # v-bias folds exactly into a constant output shift (softmax rows sum to 1)
    bv = bias_eff[:, 2 * HD :]  # [H, HD]
    out_shift = np.einsum("hd,hdf->f", bv, w_out.astype(f32)) + b_out.astype(f32)
    qkb = bias_eff[:, : 2 * HD]  # [H, 128] q/k bias
    qk_bias = bool(np.any(qkb != 0.0))
    m2 = np.asarray(mask)[0, 0]
    causal = bool(np.array_equal(m2, np.tril(np.ones((S, S), dtype=bool))))
    return wq, qkb, qk_bias, out_shift, causal, m2


def kernel(x, ln_scale, ln_bias, w_qkv, b_qkv, w_out, b_out, mask, _trace=False):
    from concourse.bass_utils import run_bass_kernel_spmd

    wq, qkb, qk_bias, out_shift, causal, m2 = _host_prep(
        x, ln_scale, ln_bias, w_qkv, b_qkv, w_out, b_out, mask
    )

    key = (causal, qk_bias)
    if key not in _cache:
        _cache[key] = _build(causal, qk_bias)
    nc = _cache[key]

    tri = np.triu(np.ones((P, P), dtype=np.float32)).astype(_BF16)  # tri[k, q] = (k <= q)
    in_maps = []
    for core in range(NCORES):
        b = core // 4
        h0 = GH * (core % 4)
        xT = np.ascontiguousarray(x[b].T).astype(_BF16)  # [D, S]
        # [D, 4, 64] -> pairs of heads packed along a 128-wide feature dim
        wq_p = np.ascontiguousarray(
            wq[:, h0 : h0 + GH, 0:HD]
            .reshape(D, 2, P)
            .reshape(DC, P, 2, P)
            .transpose(1, 0, 2, 3)
        ).astype(_BF16)
        wk_p = np.ascontiguousarray(
            wq[:, h0 : h0 + GH, HD : 2 * HD]
            .reshape(D, 2, P)
            .reshape(DC, P, 2, P)
            .transpose(1, 0, 2, 3)
        ).astype(_BF16)
        wv_p = np.ascontiguousarray(
            wq[:, h0 : h0 + GH, 2 * HD :]
            .reshape(DC, P, GH, HD)
            .transpose(1, 0, 2, 3)
        ).astype(_BF16)
        wo_p = np.ascontiguousarray(
            w_out[h0 : h0 + GH]
            .reshape(2, P, D)
            .transpose(1, 0, 2)
        ).astype(_BF16)
        m = {"xT": xT, "wq": wq_p, "wk": wk_p, "wv": wv_p, "wo": wo_p, "tri": tri}
        if qk_bias:
            qb = qkb[h0 : h0 + GH, 0:HD].reshape(2, P)
            kb = qkb[h0 : h0 + GH, HD : 2 * HD].reshape(2, P)
            m["qkb"] = np.stack([qb, kb])[None].astype(_BF16)  # [1, 2, 2, P]
        if not causal:
            m["maskT"] = np.ascontiguousarray(m2.T).astype(_BF16)
        in_maps.append(m)

    res = run_bass_kernel_spmd(nc, in_maps, list(range(NCORES)), trace=_trace)
    global last_result
    last_result = res
    out = np.zeros((B, S, D), np.float32)
    for core in range(NCORES):
        out[core // 4] += res.results[core]["out"]
    out += out_shift[None, None, :].astype(np.float32)
    if _trace:
        print(f"HW exec time: {res.exec_time_ns} ns")
    return out


# revision 37
# speedup vs baseline: 1.0997x; 1.0997x over previous
"""Trainium2 Bass kernel: causal pre-LN attention block, SPMD on 8 NeuronCores.

Sharding: core c handles batch b = c//4 and heads [4*(c%4), 4*(c%4)+4).
Each core computes LN + QKV projection for its heads + causal attention +
a partial output projection (contracting only its heads); the host sums the
4 partials per batch (the tensor-parallel reduce) and adds b_out.

Everything on device is kept "transposed" (features on partitions) so no
on-device transposes are needed:
  xT [D, S] -> LN stats via PE matmul-with-ones -> xn (bf16)
  qT,kT [64, S] per head (features on partitions), v [S, 64] (tokens on
  partitions, with a ones column appended so the attention matmul also
  produces softmax denominators)
  S^T = kT.T @ qT  [k, q]  (fp32r), exp on ACT (scores are ~N(0,1) after
  LN so no max subtraction is needed; causal masking via narrowed matmuls
  + memset + a triangular multiply)
  attnU^T = v_aug.T @ P^T accumulated in PSUM; normalize with broadcast 1/l
  out_partial = A^T.T @ w_out (fp32r)
"""

import sys

import numpy as np

sys.path.insert(0, "/opt/trn_rl_repo")

import ml_dtypes  # noqa: E402

B, S, D, H, HD = 2, 2048, 1024, 16, 64
LN_EPS = 1e-6
NCORES = 8
GH = 4  # heads per core
DC = 8  # 128-row chunks of D
ST = 16  # 128-row s tiles
QT = 4  # 512-wide q tiles
P = 128
NQ = 512

_BF16 = ml_dtypes.bfloat16

_cache: dict = {}


def _build(causal: bool, qk_bias: bool, tri_on_gpsimd: bool = False,
           exp_batched: bool = True, av_narrow: bool = True, fast_recip: bool = False):
    from contextlib import ExitStack

    import concourse.bacc as bacc
    import concourse.bass as bass
    import concourse.tile as tile
    from concourse import library_config, mybir

    f32 = mybir.dt.float32
    f32r = mybir.dt.float32r
    bf16 = mybir.dt.bfloat16
    AF = mybir.ActivationFunctionType
    OP = mybir.AluOpType

    def bcast(src_ap, n):
        """Partition-broadcast AP: replicate a 1-partition source across n."""
        return bass.AP(
            tensor=src_ap.tensor,
            offset=src_ap.offset,
            ap=[[0, n]] + [list(e) for e in src_ap.ap[1:]],
        )

    nc = bacc.Bacc("TRN2", target_bir_lowering=False, debug=False)
    d_xT = nc.declare_dram_parameter("xT", [D, S], bf16, isOutput=False)
    # q (and k) weights packed two-heads-per-128 so that for head h both qT
    # and kT live at partitions [(h%2)*64, (h%2)*64+64) (PE needs equal base
    # partitions for both matmul operands).
    d_wq = nc.declare_dram_parameter("wq", [P, DC, 2, P], bf16, isOutput=False)
    d_wk = nc.declare_dram_parameter("wk", [P, DC, 2, P], bf16, isOutput=False)
    d_wv = nc.declare_dram_parameter("wv", [P, DC, GH, HD], bf16, isOutput=False)
    d_wo = nc.declare_dram_parameter("wo", [P, 2, D], bf16, isOutput=False)
    d_tri = nc.declare_dram_parameter("tri", [P, P], bf16, isOutput=False)
    if qk_bias:
        d_qkb = nc.declare_dram_parameter("qkb", [1, 2, 2, P], bf16, isOutput=False)
    if not causal:
        d_maskT = nc.declare_dram_parameter("maskT", [S, S], bf16, isOutput=False)
    d_out = nc.declare_dram_parameter("out", [S, D], f32, isOutput=True)

    mask_eng = nc.gpsimd if tri_on_gpsimd else nc.vector

    with tile.TileContext(nc) as tc, ExitStack() as ctx:
        con = ctx.enter_context(tc.tile_pool(name="con", bufs=1))
        ps_sc = ctx.enter_context(tc.tile_pool(name="ps_sc", bufs=2, space="PSUM"))
        ps_acc = ctx.enter_context(tc.tile_pool(name="ps_acc", bufs=2, space="PSUM"))
        ps_mm = ctx.enter_context(tc.tile_pool(name="ps_mm", bufs=2, space="PSUM"))
        x2p = ctx.enter_context(tc.tile_pool(name="x2p", bufs=2))
        rowp = ctx.enter_context(tc.tile_pool(name="rowp", bufs=2))
        pp = ctx.enter_context(tc.tile_pool(name="pp", bufs=4))
        rlp = ctx.enter_context(tc.tile_pool(name="rlp", bufs=2))
        rlsp = ctx.enter_context(tc.tile_pool(name="rlsp", bufs=2))
        outp = ctx.enter_context(tc.tile_pool(name="outp", bufs=2))
        drp = ctx.enter_context(tc.tile_pool(name="drp", bufs=2, space="DRAM"))
        mkp = (
            ctx.enter_context(tc.tile_pool(name="mkp", bufs=3))
            if not causal
            else None
        )

        # ---- persistent tiles ----
        xt_sb = con.tile([P, DC, S], bf16, tag="xt")
        xn_sb = con.tile([P, DC, S], bf16, tag="xn")
        wq_sb = con.tile([P, DC, 2, P], bf16, tag="wq")
        wk_sb = con.tile([P, DC, 2, P], bf16, tag="wk")
        wv_sb = con.tile([P, DC, GH, HD], bf16, tag="wv")
        wo_sb = con.tile([P, 2, D], bf16, tag="wo")
        tri_sb = con.tile([P, P], bf16, tag="tri")
        qT_sb = con.tile([P, 2, S], bf16, tag="qT")
        kT_sb = con.tile([P, 2, S], bf16, tag="kT")
        v_sb = con.tile([P, ST, GH, HD + 1], bf16, tag="v")
        a_sb = con.tile([P, 2, S], bf16, tag="a")
        ashp = ctx.enter_context(tc.tile_pool(name="ashp", bufs=2))
        mu_b = con.tile([P, S], bf16, tag="mu_b")
        rstd_b = con.tile([P, S], bf16, tag="rstd_b")
        ones_sb = con.tile([P, 1], bf16, tag="ones")
        onesd_sb = con.tile([P, NQ], f32, tag="onesd")
        eps_sb = con.tile([1, 1], f32, tag="eps")
        if qk_bias:
            onesrow_sb = con.tile([1, NQ], bf16, tag="onesrow")
            qkb_sb = con.tile([1, 2, 2, P], bf16, tag="qkb")

        nc.gpsimd.load_library(library_config.attn)
        nc.vector.memset(ones_sb[:, :], 1.0)
        nc.vector.memset(onesd_sb[:, :], 1.0)
        nc.vector.memset(eps_sb[:, :], LN_EPS)
        # fill v wholesale with ones; v evictions overwrite cols [0:HD], leaving
        # a ones column at HD so the attention matmul also produces l
        nc.vector.memset(v_sb[:, :, :, :], 1.0)
        if qk_bias:
            nc.vector.memset(onesrow_sb[:, :], 1.0)
            nc.sync.dma_start(out=qkb_sb[:, :, :, :], in_=d_qkb[:, :, :, :])

        # ---- weight + const loads ----
        nc.sync.dma_start(out=tri_sb[:, :], in_=d_tri[:, :])
        nc.sync.dma_start(out=wq_sb[:, :, :, :], in_=d_wq[:, :, :, :])
        nc.sync.dma_start(out=wk_sb[:, :, :, :], in_=d_wk[:, :, :, :])
        nc.sync.dma_start(out=wv_sb[:, :, :, :], in_=d_wv[:, :, :, :])
        nc.sync.dma_start(out=wo_sb[:, :, :], in_=d_wo[:, :, :])

        # ---- xT loads, chunk-major, split across two DMA paths ----
        xT_r = d_xT.rearrange("(c p) s -> p c s", p=P)
        for c in range(DC):
            eng = (nc.sync, nc.gpsimd, nc.sync, nc.gpsimd)[c % 4]
            eng.dma_start(out=xt_sb[:, c, :], in_=xT_r[:, c, :])

        # ---- LN stats per 512-col slice: mu, rstd rows + partition broadcast ----
        for q4 in range(QT):
            sl = slice(q4 * NQ, (q4 + 1) * NQ)
            mu_ps = ps_acc.tile([1, NQ], f32, tag="acc")
            msq_ps = ps_acc.tile([1, NQ], f32, tag="acc")
            for c in range(DC):
                x2t = x2p.tile([P, NQ], bf16, tag="x2")
                nc.vector.tensor_mul(x2t[:, :], xt_sb[:, c, sl], xt_sb[:, c, sl])
                nc.tensor.matmul(
                    mu_ps[:, :], ones_sb[:, :], xt_sb[:, c, sl],
                    start=(c == 0), stop=(c == DC - 1),
                )
                nc.tensor.matmul(
                    msq_ps[:, :], ones_sb[:, :], x2t[:, :],
                    start=(c == 0), stop=(c == DC - 1),
                )
            # mu row (bf16) and rstd row (f32 chain then bf16)
            nc.scalar.activation(mu_b[0:1, sl], mu_ps[:, :], AF.Copy, scale=1.0 / D)
            r0 = rowp.tile([1, NQ], f32, tag="r0")
            mu2 = rowp.tile([1, NQ], f32, tag="mu2")
            nc.scalar.activation(r0[:, :], msq_ps[:, :], AF.Copy, scale=1.0 / D)
            nc.vector.tensor_mul(mu2[:, :], mu_b[0:1, sl], mu_b[0:1, sl])
            nc.vector.tensor_sub(r0[:, :], r0[:, :], mu2[:, :])
            nc.scalar.activation(r0[:, :], r0[:, :], AF.Sqrt, bias=eps_sb[:, :])
            nc.vector.reciprocal(r0[:, :], r0[:, :])
            nc.scalar.activation(rstd_b[0:1, sl], r0[:, :], AF.Copy)
            # broadcast row 0 across remaining partitions (step-0 partition
            # APs are only legal on the DRAM side, so bounce through DRAM)
            nc.gpsimd.partition_broadcast(mu_b[:, sl], mu_b[0:1, sl])
            nc.gpsimd.partition_broadcast(rstd_b[:, sl], rstd_b[0:1, sl])
            # xn = (xT - mu) * rstd for this slice (all chunks)
            for c in range(DC):
                nc.vector.tensor_sub(xn_sb[:, c, sl], xt_sb[:, c, sl], mu_b[:, sl])
                nc.vector.tensor_mul(xn_sb[:, c, sl], xn_sb[:, c, sl], rstd_b[:, sl])

        def emit_outproj(qt, only_st=None):
            for st in ([only_st] if only_st is not None else range(4 * qt, 4 * qt + 4)):
                ssl = slice(st * P, (st + 1) * P)
                for fb in range(2):
                    fsl = slice(fb * NQ, (fb + 1) * NQ)
                    o_ps = ps_mm.tile([P, NQ], f32, tag="mm")
                    for c2 in range(2):
                        nc.tensor.matmul(
                            o_ps[:, :], a_sb[:, c2, ssl], wo_sb[:, c2, fsl],
                            start=(c2 == 0), stop=(c2 == 1),
                        )
                    ot = outp.tile([P, NQ], f32, tag="out")
                    nc.vector.tensor_copy(ot[:, :], o_ps[:, :])
                    nc.sync.dma_start(out=d_out[ssl, fsl], in_=ot[:, :])

        # ---- main loop over q blocks ----
        for qt in range(QT):
            qsl = slice(qt * NQ, (qt + 1) * NQ)
            # v for the 4 s-tiles of this block
            for st in range(4 * qt, 4 * qt + 4):
                ssl = slice(st * P, (st + 1) * P)
                v_ps = ps_mm.tile([P, GH, HD], f32, tag="mm")
                for c in range(DC):
                    nc.tensor.matmul(
                        v_ps[:, :, :], xn_sb[:, c, ssl], wv_sb[:, c, :, :],
                        start=(c == 0), stop=(c == DC - 1),
                    )
                nc.scalar.activation(v_sb[:, st, :, 0:HD], v_ps[:, :, :], AF.Copy)
            # qT/kT, two heads packed per matmul (pair pr = h//2)
            for qk in range(2):
                w_sb, dst = (wq_sb, qT_sb) if qk == 0 else (wk_sb, kT_sb)
                for pr in range(2):
                    qk_ps = ps_mm.tile([P, NQ], f32, tag="mm")
                    for c in range(DC):
                        nc.tensor.matmul(
                            qk_ps[:, :], w_sb[:, c, pr, :], xn_sb[:, c, qsl],
                            start=(c == 0), stop=(c == DC - 1) and not qk_bias,
                        )
                    if qk_bias:
                        nc.tensor.matmul(
                            qk_ps[:, :], qkb_sb[:, qk, pr, :], onesrow_sb[:, :],
                            start=False, stop=True,
                        )
                    nc.vector.tensor_copy(dst[:, pr, qsl], qk_ps[:, :])

            # attention for each head
            for h in range(GH):
                even = h % 2 == 0
                pr = h // 2
                r0h = (h % 2) * HD  # partition base of this head's qT/kT rows
                acc = ps_acc.tile([P, NQ], f32, tag="acc")
                kts = list(range(4 * qt + 4)) if causal else list(range(ST))
                n_kt = len(kts)
                c0s = {}
                for g0 in range(0, n_kt, 2):
                    grp = kts[g0 : g0 + 2]
                    st_ps = ps_sc.tile([P, 2, NQ], f32, tag="sc")
                    pt = pp.tile([P, 2, NQ], bf16, tag="p")
                    mt = mkp.tile([P, 2, NQ], bf16, tag="mk") if not causal else None
                    for i, kt in enumerate(grp):
                        j = kt - 4 * qt  # >=0 on the diagonal band (causal)
                        ksl = slice(kt * P, (kt + 1) * P)
                        if causal and j >= 0:
                            c0 = min(j * P, 2 * P)  # keep moving width >= 256
                        else:
                            c0 = 0
                        c0s[kt] = c0
                        nc.tensor.matmul(
                            st_ps[:, i, c0:NQ],
                            kT_sb[r0h : r0h + HD, pr, ksl],
                            qT_sb[r0h : r0h + HD, pr, qt * NQ + c0 : (qt + 1) * NQ],
                            start=True, stop=True,
                        )
                    # one exp over the whole group; garbage in the unused
                    # region [0:c0] is never read by the narrowed AV matmul
                    if exp_batched:
                        nc.scalar.activation(
                            pt[:, :, :], st_ps[:, :, :], AF.Exp, scale=0.125
                        )
                    else:
                        for i, kt in enumerate(grp):
                            c0 = c0s[kt]
                            nc.scalar.activation(
                                pt[:, i, c0:NQ], st_ps[:, i, c0:NQ], AF.Exp, scale=0.125
                            )
                    if not causal:
                        nc.sync.dma_start(
                            out=mt[:, :, :],
                            in_=d_maskT.rearrange("(kt p) s -> p kt s", p=P)[
                                :, grp[0] : grp[0] + 2, qsl
                            ],
                        )
                        nc.vector.tensor_mul(pt[:, :, :], pt[:, :, :], mt[:, :, :])
                    for i, kt in enumerate(grp):
                        j = kt - 4 * qt
                        if causal and j >= 0:
                            m0 = j * P  # first valid column of this band block
                            if m0 > c0s[kt]:
                                nc.vector.memset(pt[:, i, c0s[kt] : m0], 0.0)
                            mask_eng.tensor_mul(
                                pt[:, i, m0 : m0 + P], pt[:, i, m0 : m0 + P],
                                tri_sb[:, :],
                            )
                    for i, kt in enumerate(grp):
                        c0 = c0s[kt] if av_narrow else 0
                        if not av_narrow and c0s[kt] < (kt - 4 * qt) * P:
                            nc.vector.memset(pt[:, i, c0s[kt] : (kt - 4 * qt) * P], 0.0)
                        nc.tensor.matmul(
                            acc[0 : HD + 1, c0:NQ], v_sb[:, kt, h, :], pt[:, i, c0:NQ],
                            start=(kt == kts[0]), stop=(kt == kts[-1]),
                        )
                # normalize: rl = 1/l broadcast across the 64 head dims
                rls = rlsp.tile([P, 2, NQ], f32, tag="rls")
                rl_b = rlp.tile([P, NQ], f32, tag="rlb")
                lrow = slice(HD, HD + 1)
                nc.vector.tensor_copy(rls[lrow, 0, :], acc[lrow, :])
                rrows = slice(0, HD)
                nc.gpsimd.partition_broadcast(rl_b[rrows, :], rls[lrow, 0, :])
                nc.vector.reciprocal(rl_b[rrows, :], rl_b[rrows, :])
                if even:
                    nc.vector.tensor_mul(
                        a_sb[0:HD, h // 2, qsl], acc[0:HD, :], rl_b[0:HD, :]
                    )
                else:
                    # engines cannot shift partitions; bounce through a DMA
                    ash = ashp.tile([HD, NQ], bf16, tag="ash")
                    nc.vector.tensor_mul(ash[:, :], acc[0:HD, :], rl_b[0:HD, :])
                    nc.gpsimd.dma_start(
                        out=a_sb[HD : 2 * HD, h // 2, qsl], in_=ash[:, :]
                    )
                # deferred output projection piece: fills PE stalls while the
                # next head's attention waits on exp
                if qt >= 1:
                    emit_outproj(qt - 1, only_st=4 * (qt - 1) + h)

        emit_outproj(QT - 1)

    nc.compile()
    return nc


def _host_prep(x, ln_scale, ln_bias, w_qkv, b_qkv, w_out, b_out, mask):
    """Fold LN affine params and exactly-absorbable biases on the host."""
    f32 = np.float32
    wq = (ln_scale.astype(f32)[:, None, None] * w_qkv.astype(f32))  # [D, H, 192]
    bias_eff = ln_bias.astype(f32) @ w_qkv.reshape(D, -1).astype(f32)
    bias_eff = bias_eff.reshape(H, 3 * HD) + b_qkv.astype(f32)  # [H, 192]
    # v-bias folds exactly into a constant output shift (softmax rows sum to 1)
    bv = bias_eff[:, 2 * HD :]  # [H, HD]
    out_shift = np.einsum("hd,hdf->f", bv, w_out.astype(f32)) + b_out.astype(f32)
    qkb = bias_eff[:, : 2 * HD]  # [H, 128] q/k bias
    qk_bias = bool(np.any(qkb != 0.0))
    m2 = np.asarray(mask)[0, 0]
    causal = bool(np.array_equal(m2, np.tril(np.ones((S, S), dtype=bool))))
    return wq, qkb, qk_bias, out_shift, causal, m2


def kernel(x, ln_scale, ln_bias, w_qkv, b_qkv, w_out, b_out, mask, _trace=False):
    from concourse.bass_utils import run_bass_kernel_spmd

    wq, qkb, qk_bias, out_shift, causal, m2 = _host_prep(
        x, ln_scale, ln_bias, w_qkv, b_qkv, w_out, b_out, mask
    )

    key = (causal, qk_bias)
    if key not in _cache:
        _cache[key] = _build(causal, qk_bias)
    nc = _cache[key]

    tri = np.triu(np.ones((P, P), dtype=np.float32)).astype(_BF16)  # tri[k, q] = (k <= q)
    in_maps = []
    for core in range(NCORES):
        b = core // 4
        h0 = GH * (core % 4)
        xT = np.ascontiguousarray(x[b].T).astype(_BF16)  # [D, S]
        # [D, 4, 64] -> pairs of heads packed along a 128-wide feature dim
        wq_p = np.ascontiguousarray(
            wq[:, h0 : h0 + GH, 0:HD]
            .reshape(D, 2, P)
            .reshape(DC, P, 2, P)
            .transpose(1, 0, 2, 3)
        ).astype(_BF16)
        wk_p = np.ascontiguousarray(
            wq[:, h0 : h0 + GH, HD : 2 * HD]
            .reshape(D, 2, P)
            .reshape(DC, P, 2, P)
            .transpose(1, 0, 2, 3)
        ).astype(_BF16)
        wv_p = np.ascontiguousarray(
            wq[:, h0 : h0 + GH, 2 * HD :]
            .reshape(DC, P, GH, HD)
            .transpose(1, 0, 2, 3)
        ).astype(_BF16)
        wo_p = np.ascontiguousarray(
            w_out[h0 : h0 + GH]
            .reshape(2, P, D)
            .transpose(1, 0, 2)
        ).astype(_BF16)
        m = {"xT": xT, "wq": wq_p, "wk": wk_p, "wv": wv_p, "wo": wo_p, "tri": tri}
        if qk_bias:
            qb = qkb[h0 : h0 + GH, 0:HD].reshape(2, P)
            kb = qkb[h0 : h0 + GH, HD : 2 * HD].reshape(2, P)
            m["qkb"] = np.stack([qb, kb])[None].astype(_BF16)  # [1, 2, 2, P]
        if not causal:
            m["maskT"] = np.ascontiguousarray(m2.T).astype(_BF16)
        in_maps.append(m)

    res = run_bass_kernel_spmd(nc, in_maps, list(range(NCORES)), trace=_trace)
    global last_result
    last_result = res
    out = np.zeros((B, S, D), np.float32)
    for core in range(NCORES):
        out[core // 4] += res.results[core]["out"]
    out += out_shift[None, None, :].astype(np.float32)
    if _trace:
        print(f"HW exec time: {res.exec_time_ns} ns")
    return out
